# revision 14
# baseline (speedup 1.0000x reference)
"""Trainium2 Bass kernel for CompanyOperationEvaluation ('rec' branch).

Data-parallel over batch across 8 NeuronCores. Embedding tables and MLP
weights are replicated; features/ent_idx are sharded along B. All matmuls
run in bf16 (1 cyc/row on the PE, reorderable LDWEIGHTS) with activations
kept transposed ([feature, batch]) so weights serve as lhsT in their
natural [in, out] layout; PSUM accumulation stays fp32.

The cross-compress recurrence is collapsed algebraically: with
h1 = a1*h0 + b1*e0 + b_c and e1 = g1*h0 + d1*e0 + b_e (per-row scalars from
dot products), the only tensor the MLP head needs is
e2 = A*h0 + B*e0 + C, where A, B, C derive from six per-row dot products
(h0/e0 against w_cf/w_ef/w_fe/w_fc) plus column sums of w_ef/w_fe.

Scheduling notes: the PE executes its stream in order. The per-128-row
embedding gathers (one 1KB-descriptor indirect DMA per 128 rows, head/ent
interleaved host-side) are the longest serial resource (~1.1us each on the
GpSimd descriptor generator), so per-group cross-compress chains are
interleaved with the later MLP stages of already-finished groups to keep
the PE fed while gathers stream in.
"""

import numpy as np

B, F, D = 16384, 256, 128
H0, H1, OUT = 512, 256, 8
VOCAB = 100000
NCORES = 8
BC = B // NCORES       # 2048 rows per core
NT = BC // 128         # 16 tiles of 128 rows
NG = BC // 512         # 4 groups of 512 rows
GT = 512 // 128        # 4 tiles per group

_CACHE = {}


def _build():
    import concourse.bacc as bacc
    import concourse.bass as bass
    import concourse.tile as tile
    from concourse import mybir

    f32 = mybir.dt.float32
    bf16 = mybir.dt.bfloat16
    i32 = mybir.dt.int32
    AF = mybir.ActivationFunctionType
    OP = mybir.AluOpType
    AX = mybir.AxisListType

    nc = bacc.Bacc()

    featT = nc.dram_tensor("featT", (F, BC), bf16, kind="ExternalInput")
    # idx2[p, t] = ent_idx[t*128 + p]
    idx2 = nc.dram_tensor("idx2", (128, NT), i32, kind="ExternalInput")
    tabs = nc.dram_tensor("tabs", (VOCAB, 2 * D), f32, kind="ExternalInput")
    identb_d = nc.dram_tensor("identb", (128, 128), bf16, kind="ExternalInput")
    # wp1 = [wf(2x128) | wu(128) | whe(6)]; wp2 = [w0(2x512) | w1(4x256) | w2(2x8)]
    wp1 = nc.dram_tensor("wp1", (128, 2 * D + D + 6), bf16, kind="ExternalInput")
    wp2 = nc.dram_tensor("wp2", (128, 2 * H0 + 4 * H1 + 2 * OUT), bf16,
                         kind="ExternalInput")
    # biases packed: [bfu(2) | b0r(4) | b1r(2) | bce(2) | b2(rows 0..7 of col 10)]
    bp = nc.dram_tensor("bp", (128, 11), f32, kind="ExternalInput")
    prob = nc.dram_tensor("prob", (BC, OUT), f32, kind="ExternalOutput")

    with tile.TileContext(nc) as tc:
        with (
            tc.tile_pool(name="pers", bufs=1) as pers,
            tc.tile_pool(name="work", bufs=3) as work,
            tc.tile_pool(name="ps", bufs=4, space="PSUM") as psp,
        ):
            # ---- DMAs, earliest-needed first (triggers are ~0.7us apiece) ----
            ix_t = pers.tile([128, NT], i32, tag="ix")
            nc.sync.dma_start(out=ix_t[:], in_=idx2[:])
            he_all = pers.tile([128, NT * 2 * D], bf16, tag="he")
            for t in range(NT):
                nc.gpsimd.indirect_dma_start(
                    out=he_all[:, t * 2 * D:(t + 1) * 2 * D],
                    out_offset=None, in_=tabs[:],
                    in_offset=bass.IndirectOffsetOnAxis(ap=ix_t[:, t:t + 1], axis=0))
            xT = pers.tile([128, F // 128, BC], bf16, tag="xT")
            fv = featT.rearrange("(a p) b -> p a b", p=128)
            nc.sync.dma_start(out=xT[:, 0, :], in_=fv[:, 0, :])
            wp1_t = pers.tile([128, 2 * D + D + 6], bf16, tag="wp1")
            nc.sync.dma_start(out=wp1_t[:], in_=wp1[:])
            nc.sync.dma_start(out=xT[:, 1, :], in_=fv[:, 1, :])
            wf_t = wp1_t[:, 0:2 * D].rearrange("p (a d) -> p a d", d=D)
            wu_t = wp1_t[:, 2 * D:3 * D]
            whe_t = wp1_t[:, 3 * D:3 * D + 6]
            identb = pers.tile([128, 128], bf16, tag="identb")
            nc.sync.dma_start(out=identb[:], in_=identb_d[:])
            bp_t = pers.tile([128, 11], f32, tag="bp")
            nc.sync.dma_start(out=bp_t[:], in_=bp[:])
            bfu_t = bp_t[:, 0:2]
            b0_t = bp_t[:, 2:6]
            b1_t = bp_t[:, 6:8]
            bce_t = bp_t[:, 8:10]
            b2_t = bp_t[:OUT, 10:11]
            wp2_t = pers.tile([128, 2 * H0 + 4 * H1 + 2 * OUT], bf16, tag="wp2")
            nc.sync.dma_start(out=wp2_t[:], in_=wp2[:])
            w0_t = wp2_t[:, 0:2 * H0].rearrange("p (a h) -> p a h", h=H0)
            w1_t = wp2_t[:, 2 * H0:2 * H0 + 4 * H1].rearrange(
                "p (a h) -> p a h", h=H1)
            w2_t = wp2_t[:, 2 * H0 + 4 * H1:].rearrange("p (a o) -> p a o", o=OUT)

            ones_t = pers.tile([128, 128], bf16, tag="ones_t")
            nc.vector.memset(ones_t[:], 1.0)

            # ---- column sums of wHE, broadcast to all partitions ----
            ones_c = pers.tile([128, 1], bf16, tag="ones_c")
            nc.vector.memset(ones_c[:], 1.0)
            ones_r = pers.tile([1, 128], bf16, tag="ones_r")
            nc.vector.memset(ones_r[:], 1.0)
            sums_ps = psp.tile([1, 6], f32, tag="smps", bufs=2)
            nc.tensor.matmul(out=sums_ps[:], lhsT=ones_c[:], rhs=whe_t,
                             start=True, stop=True)
            sums_sb = pers.tile([1, 6], bf16, tag="sums")
            nc.scalar.activation(out=sums_sb[:], in_=sums_ps[:], func=AF.Copy)
            sb_ps = psp.tile([128, 6], f32, tag="smps", bufs=2)
            nc.tensor.matmul(out=sb_ps[:], lhsT=ones_r[:], rhs=sums_sb[:],
                             start=True, stop=True)
            sb_t = pers.tile([128, 6], bf16, tag="sb")
            nc.scalar.activation(out=sb_t[:], in_=sb_ps[:], func=AF.Copy)
            # cef = b_e * sum(w_ef); cfe = b_c * sum(w_fe)
            cef = pers.tile([128, 1], f32, tag="cef")
            nc.vector.tensor_tensor(out=cef[:], in0=sb_t[:, 2:3], in1=bce_t[:, 1:2],
                                    op=OP.mult)
            cfe = pers.tile([128, 1], f32, tag="cfe")
            nc.vector.tensor_tensor(out=cfe[:], in0=sb_t[:, 1:2], in1=bce_t[:, 0:1],
                                    op=OP.mult)

            def h0s(t):
                return he_all[:, t * 2 * D:t * 2 * D + D]

            def e0s(t):
                return he_all[:, t * 2 * D + D:(t + 1) * 2 * D]

            hT_all = pers.tile([128, BC], bf16, tag="hT")
            eT_all = pers.tile([128, BC], bf16, tag="eT")
            dots = pers.tile([128, NT * 8], bf16, tag="dots")
            A = pers.tile([128, NT], f32, tag="A")
            Bc = pers.tile([128, NT], f32, tag="B")
            Cc = pers.tile([128, NT], f32, tag="C")
            e2T = pers.tile([128, BC], bf16, tag="e2T")

            def tt(out, a, b, op):
                nc.vector.tensor_tensor(out=out, in0=a, in1=b, op=op)

            def cross(g):
                """transpose h0/e0, dots, coefficients, e2 for one 512-group."""
                gs = slice(g * 512, (g + 1) * 512)
                for sel, dst in ((h0s, hT_all), (e0s, eT_all)):
                    tp = psp.tile([128, 512], bf16, tag="trps", bufs=2)
                    for j in range(GT):
                        nc.tensor.transpose(out=tp[:, j * 128:(j + 1) * 128],
                                            in_=sel(g * GT + j), identity=identb[:])
                    nc.scalar.activation(out=dst[:, gs], in_=tp[:], func=AF.Copy)
                # dots: 0..3 = h0.(w_fc,w_fe,w_ef,w_cf); 4..7 = e0.(w_ef,w_cf,w_ef,w_fe)
                for j in range(GT):
                    t = g * GT + j
                    bs = slice(t * 128, (t + 1) * 128)
                    d_ps = psp.tile([128, 8], f32, tag="smps", bufs=2)
                    nc.tensor.matmul(out=d_ps[:, 0:4], lhsT=hT_all[:, bs],
                                     rhs=whe_t[:, 0:4], start=True, stop=True)
                    nc.tensor.matmul(out=d_ps[:, 4:8], lhsT=eT_all[:, bs],
                                     rhs=whe_t[:, 2:6], start=True, stop=True)
                    nc.scalar.activation(out=dots[:, t * 8:(t + 1) * 8],
                                         in_=d_ps[:, 0:8], func=AF.Copy)
                # coefficients for this group's 4 tiles
                dv = dots.rearrange("p (t c) -> p t c", c=8)
                tg = slice(g * GT, (g + 1) * GT)
                c0, c1, c2 = dv[:, tg, 0], dv[:, tg, 1], dv[:, tg, 2]
                c3, c4, c5 = dv[:, tg, 5], dv[:, tg, 4], dv[:, tg, 7]
                t1 = work.tile([128, GT], f32, tag="t1")
                t2 = work.tile([128, GT], f32, tag="t2")
                a2 = work.tile([128, GT], f32, tag="a2")
                d2 = work.tile([128, GT], f32, tag="d2")
                # a2 = c4*c2 + c1*c4 + cef ; d2 = c3*c1 + c0*c5 + cfe
                tt(t1[:], c4, c2, OP.mult)
                tt(t2[:], c1, c4, OP.mult)
                tt(a2[:], t1[:], t2[:], OP.add)
                nc.vector.tensor_scalar(out=a2[:], in0=a2[:], scalar1=cef[:, 0:1],
                                        scalar2=None, op0=OP.add)
                tt(t1[:], c3, c1, OP.mult)
                tt(t2[:], c0, c5, OP.mult)
                tt(d2[:], t1[:], t2[:], OP.add)
                nc.vector.tensor_scalar(out=d2[:], in0=d2[:], scalar1=cfe[:, 0:1],
                                        scalar2=None, op0=OP.add)
                # A = a2*c3 + d2*c4 ; B = a2*c0 + d2*c1 ; C = a2*b_c + (d2*b_e + b_e)
                tt(t1[:], a2[:], c3, OP.mult)
                tt(t2[:], d2[:], c4, OP.mult)
                tt(A[:, tg], t1[:], t2[:], OP.add)
                tt(t1[:], a2[:], c0, OP.mult)
                tt(t2[:], d2[:], c1, OP.mult)
                tt(Bc[:, tg], t1[:], t2[:], OP.add)
                nc.vector.tensor_scalar(out=t1[:], in0=a2[:], scalar1=bce_t[:, 0:1],
                                        scalar2=None, op0=OP.mult)
                nc.vector.tensor_scalar(out=t2[:], in0=d2[:], scalar1=bce_t[:, 1:2],
                                        scalar2=bce_t[:, 1:2], op0=OP.mult, op1=OP.add)
                tt(Cc[:, tg], t1[:], t2[:], OP.add)
                # e2T = h0'.diag(A) + e0'.diag(B) + ones'.diag(C): the gathered
                # normal-layout tiles serve directly as lhsT (contraction over b)
                e2_ps = psp.tile([128, 512], f32, tag="mmps", bufs=4)
                for j in range(GT):
                    t = g * GT + j
                    js = slice(j * 128, (j + 1) * 128)
                    dga = work.tile([128, 128], bf16, tag="dga")
                    nc.scalar.activation(out=dga[:], in_=identb[:], func=AF.Copy,
                                         scale=A[:, t:t + 1])
                    dgb = work.tile([128, 128], bf16, tag="dgb")
                    nc.vector.tensor_scalar(out=dgb[:], in0=identb[:],
                                            scalar1=Bc[:, t:t + 1], scalar2=None,
                                            op0=OP.mult)
                    dgc = work.tile([128, 128], bf16, tag="dgc")
                    nc.scalar.activation(out=dgc[:], in_=identb[:], func=AF.Copy,
                                         scale=Cc[:, t:t + 1])
                    nc.tensor.matmul(out=e2_ps[:, js], lhsT=h0s(t), rhs=dga[:],
                                     start=True, stop=False)
                    nc.tensor.matmul(out=e2_ps[:, js], lhsT=e0s(t), rhs=dgb[:],
                                     start=False, stop=False)
                    nc.tensor.matmul(out=e2_ps[:, js], lhsT=ones_t[:], rhs=dgc[:],
                                     start=False, stop=True)
                nc.scalar.activation(out=e2T[:, gs], in_=e2_ps[:], func=AF.Copy)

            GS = [slice(g * 512, (g + 1) * 512) for g in range(NG)]

            def relu(dst, src, bias_ap, on_vector):
                if on_vector:
                    nc.vector.tensor_scalar(out=dst, in0=src, scalar1=bias_ap,
                                            scalar2=0.0, op0=OP.add, op1=OP.max)
                else:
                    nc.scalar.activation(out=dst, in_=src, func=AF.Relu, bias=bias_ap)

            cf0 = pers.tile([128, BC], bf16, tag="cf0")
            cf1 = pers.tile([128, BC], bf16, tag="cf1")
            cf2 = pers.tile([128, BC], bf16, tag="cf2")
            x1a = pers.tile([128, NG, 4 * 512], bf16, tag="x1a")
            x2a = pers.tile([128, NG, 2 * 512], bf16, tag="x2a")
            x3a = pers.tile([OUT, NG, 512], bf16, tag="x3a")

            def mlp0(g):
                """feature layer + 2x user mlp for one group (needs only xT)."""
                cf_ps = psp.tile([128, 512], f32, tag="mmps", bufs=4)
                nc.tensor.matmul(out=cf_ps[:], lhsT=wf_t[:, 0, :], rhs=xT[:, 0, GS[g]],
                                 start=True, stop=False)
                nc.tensor.matmul(out=cf_ps[:], lhsT=wf_t[:, 1, :], rhs=xT[:, 1, GS[g]],
                                 start=False, stop=True)
                relu(cf0[:, GS[g]], cf_ps[:], bfu_t[:, 0:1], on_vector=False)
                for src, dst in ((cf0, cf1), (cf1, cf2)):
                    cu_ps = psp.tile([128, 512], f32, tag="mmps", bufs=4)
                    nc.tensor.matmul(out=cu_ps[:], lhsT=wu_t, rhs=src[:, GS[g]],
                                     start=True, stop=True)
                    relu(dst[:, GS[g]], cu_ps[:], bfu_t[:, 1:2],
                         on_vector=(g % 2 == 1))

            def w0(g):
                for m in range(4):
                    x1_ps = psp.tile([128, 512], f32, tag="mmps", bufs=4)
                    ms = slice(m * 128, (m + 1) * 128)
                    nc.tensor.matmul(out=x1_ps[:], lhsT=w0_t[:, 0, ms],
                                     rhs=cf2[:, GS[g]], start=True, stop=False)
                    nc.tensor.matmul(out=x1_ps[:], lhsT=w0_t[:, 1, ms],
                                     rhs=e2T[:, GS[g]], start=False, stop=True)
                    relu(x1a[:, g, m * 512:(m + 1) * 512], x1_ps[:], b0_t[:, m:m + 1],
                         on_vector=(m % 2 == 1))

            def w1(g):
                for m in range(2):
                    x2_ps = psp.tile([128, 512], f32, tag="mmps", bufs=4)
                    ms = slice(m * 128, (m + 1) * 128)
                    for k in range(4):
                        nc.tensor.matmul(out=x2_ps[:], lhsT=w1_t[:, k, ms],
                                         rhs=x1a[:, g, k * 512:(k + 1) * 512],
                                         start=(k == 0), stop=(k == 3))
                    relu(x2a[:, g, m * 512:(m + 1) * 512], x2_ps[:], b1_t[:, m:m + 1],
                         on_vector=(m % 2 == 1))

            def w2sm(g):
                """pred layer + softmax + store for one group."""
                x3_ps = psp.tile([OUT, 512], f32, tag="smps", bufs=2)
                for k in range(2):
                    nc.tensor.matmul(out=x3_ps[:], lhsT=w2_t[:, k, :],
                                     rhs=x2a[:, g, k * 512:(k + 1) * 512],
                                     start=(k == 0), stop=(k == 1))
                nc.scalar.activation(out=x3a[:, g, :], in_=x3_ps[:], func=AF.Relu,
                                     bias=b2_t[:, 0:1])
                ex = work.tile([128, GT * OUT], f32, tag="ex")
                for j in range(GT):
                    sm_ps = psp.tile([128, OUT], bf16, tag="smps", bufs=2)
                    nc.tensor.transpose(out=sm_ps[:],
                                        in_=x3a[:, g, j * 128:(j + 1) * 128],
                                        identity=identb[:OUT, :OUT])
                    nc.scalar.activation(out=ex[:, j * OUT:(j + 1) * OUT],
                                         in_=sm_ps[:], func=AF.Exp)
                sm_sum = work.tile([128, GT], f32, tag="sm_sum")
                nc.vector.reduce_sum(out=sm_sum[:],
                                     in_=ex.rearrange("p (j o) -> p j o", o=OUT),
                                     axis=AX.X)
                sm_rec = work.tile([128, GT], f32, tag="sm_rec")
                nc.vector.reciprocal(out=sm_rec[:], in_=sm_sum[:])
                pr = work.tile([128, GT * OUT], f32, tag="pr")
                nc.vector.tensor_tensor(
                    out=pr.rearrange("p (j o) -> p j o", o=OUT),
                    in0=ex.rearrange("p (j o) -> p j o", o=OUT),
                    in1=sm_rec.rearrange("p (j o) -> p j o", o=1).to_broadcast(
                        [128, GT, OUT]),
                    op=OP.mult)
                nc.sync.dma_start(
                    out=prob[g * 512:(g + 1) * 512, :].rearrange(
                        "(j p) o -> p j o", p=128),
                    in_=pr.rearrange("p (j o) -> p j o", o=OUT))

            # PE program order: early group-independent matmuls first, then
            # cross-compress chains interleaved with later-stage MLP work so
            # the PE never stalls on a not-yet-gathered group.
            for g in range(NG):
                mlp0(g)
            cross(0)
            cross(1)
            w0(0)
            cross(2)
            w0(1)
            w1(0)
            cross(3)
            w0(2)
            w1(1)
            w2sm(0)
            w0(3)
            w1(2)
            w2sm(1)
            w1(3)
            w2sm(2)
            w2sm(3)

    nc.finalize()
    return nc


def _get_nc():
    if "nc" not in _CACHE:
        _CACHE["nc"] = _build()
    return _CACHE["nc"]


def kernel(features, ent_idx, target, Wf, bf, Wu, bu, w_cf, w_fc, w_ef, w_fe,
           b_c, b_e, head_tab, ent_tab, W0, b0, W1, b1, W2, b2):
    import ml_dtypes
    from concourse.bass_utils import run_bass_kernel_spmd

    bf16 = ml_dtypes.bfloat16
    f32 = np.float32
    features = np.asarray(features, dtype=f32)
    ent_idx = np.asarray(ent_idx)
    target = np.asarray(target)
    head_tab = np.asarray(head_tab, dtype=f32)
    ent_tab = np.asarray(ent_tab, dtype=f32)
    w_cf, w_fc = np.asarray(w_cf, f32), np.asarray(w_fc, f32)
    w_ef, w_fe = np.asarray(w_ef, f32), np.asarray(w_fe, f32)

    featT = np.ascontiguousarray(features.T.astype(bf16))        # [F, B]
    idx_all = ent_idx.astype(np.int32)
    tabs = np.ascontiguousarray(np.concatenate([head_tab, ent_tab], axis=1))
    wHE = np.stack([w_fc, w_fe, w_ef, w_cf, w_ef, w_fe], axis=1).astype(bf16)
    wf_p = np.asarray(Wf, f32).astype(bf16).reshape(2, D, D).transpose(
        1, 0, 2).reshape(D, 2 * D)
    wu_p = np.asarray(Wu, f32).astype(bf16)
    wp1 = np.ascontiguousarray(np.concatenate([wf_p, wu_p, wHE], axis=1))
    w0_p = np.asarray(W0, f32).astype(bf16).reshape(2, D, H0).transpose(
        1, 0, 2).reshape(D, 2 * H0)
    w1_p = np.asarray(W1, f32).astype(bf16).reshape(4, D, H1).transpose(
        1, 0, 2).reshape(D, 4 * H1)
    w2_p = np.asarray(W2, f32).astype(bf16).reshape(2, D, OUT).transpose(
        1, 0, 2).reshape(D, 2 * OUT)
    wp2 = np.ascontiguousarray(np.concatenate([w0_p, w1_p, w2_p], axis=1))
    bp = np.zeros((128, 11), f32)
    bp[:, 0] = np.asarray(bf, f32)
    bp[:, 1] = np.asarray(bu, f32)
    bp[:, 2:6] = np.asarray(b0, f32).reshape(4, D).T
    bp[:, 6:8] = np.asarray(b1, f32).reshape(2, D).T
    bp[:, 8] = np.float32(np.asarray(b_c, f32).reshape(()))
    bp[:, 9] = np.float32(np.asarray(b_e, f32).reshape(()))
    bp[:OUT, 10] = np.asarray(b2, f32)
    identb = np.eye(128, dtype=bf16)

    shared = dict(tabs=tabs, identb=identb, wp1=wp1, wp2=wp2, bp=bp)
    in_maps = []
    for c in range(NCORES):
        cs = slice(c * BC, (c + 1) * BC)
        in_maps.append(dict(
            featT=np.ascontiguousarray(featT[:, cs]),
            idx2=np.ascontiguousarray(idx_all[cs].reshape(NT, 128).T),
            **shared))

    nc = _get_nc()
    res = run_bass_kernel_spmd(nc, in_maps, core_ids=list(range(NCORES)))
    prob = np.concatenate([r["prob"] for r in res.results], axis=0)
    return prob, target


# revision 17
# speedup vs baseline: 1.0434x; 1.0434x over previous
"""Trainium2 Bass kernel for CompanyOperationEvaluation ('rec' branch).

Data-parallel over batch across 8 NeuronCores. Embedding tables and MLP
weights are replicated; features/ent_idx are sharded along B. All matmuls
run in bf16 (1 cyc/row on the PE, reorderable LDWEIGHTS) with activations
kept transposed ([feature, batch]) so weights serve as lhsT in their
natural [in, out] layout; PSUM accumulation stays fp32.

The cross-compress recurrence is collapsed algebraically: with
h1 = a1*h0 + b1*e0 + b_c and e1 = g1*h0 + d1*e0 + b_e (per-row scalars from
dot products), the only tensor the MLP head needs is
e2 = A*h0 + B*e0 + C, where A, B, C derive from six per-row dot products
(h0/e0 against w_cf/w_ef/w_fe/w_fc) plus column sums of w_ef/w_fe.

Scheduling notes: the PE executes its stream in order. The per-128-row
embedding gathers (one 1KB-descriptor indirect DMA per 128 rows, head/ent
interleaved host-side) are the longest serial resource (~1.1us each on the
GpSimd descriptor generator), so per-group cross-compress chains are
interleaved with the later MLP stages of already-finished groups to keep
the PE fed while gathers stream in.
"""

import numpy as np

B, F, D = 16384, 256, 128
H0, H1, OUT = 512, 256, 8
VOCAB = 100000
NCORES = 8
BC = B // NCORES       # 2048 rows per core
NT = BC // 128         # 16 tiles of 128 rows
NG = BC // 512         # 4 groups of 512 rows
GT = 512 // 128        # 4 tiles per group

_CACHE = {}


def _build():
    import concourse.bacc as bacc
    import concourse.bass as bass
    import concourse.tile as tile
    from concourse import mybir

    f32 = mybir.dt.float32
    bf16 = mybir.dt.bfloat16
    i32 = mybir.dt.int32
    AF = mybir.ActivationFunctionType
    OP = mybir.AluOpType
    AX = mybir.AxisListType

    nc = bacc.Bacc()

    featT = nc.dram_tensor("featT", (F, BC), bf16, kind="ExternalInput")
    # idx2[p, t] = ent_idx[t*128 + p]
    idx2 = nc.dram_tensor("idx2", (128, NT), i32, kind="ExternalInput")
    tabs = nc.dram_tensor("tabs", (VOCAB, 2 * D), f32, kind="ExternalInput")
    identb_d = nc.dram_tensor("identb", (128, 128), bf16, kind="ExternalInput")
    # wp1 = [wf(2x128) | wu(128) | whe(6)]; wp2 = [w0(2x512) | w1(4x256) | w2(2x8)]
    wp1 = nc.dram_tensor("wp1", (128, 2 * D + D + 6), bf16, kind="ExternalInput")
    wp2 = nc.dram_tensor("wp2", (128, 2 * H0 + 4 * H1 + 2 * OUT), bf16,
                         kind="ExternalInput")
    # biases packed: [bfu(2) | b0r(4) | b1r(2) | bce(2) | b2(rows 0..7 of col 10)]
    bp = nc.dram_tensor("bp", (128, 11), f32, kind="ExternalInput")
    prob = nc.dram_tensor("prob", (BC, OUT), f32, kind="ExternalOutput")

    with tile.TileContext(nc) as tc:
        with (
            tc.tile_pool(name="pers", bufs=1) as pers,
            tc.tile_pool(name="work", bufs=3) as work,
            tc.tile_pool(name="ps", bufs=4, space="PSUM") as psp,
        ):
            # ---- DMAs, earliest-needed first (triggers are ~0.7us apiece) ----
            ix_t = pers.tile([128, NT], i32, tag="ix")
            nc.gpsimd.dma_start(out=ix_t[:], in_=idx2[:])
            he_all = pers.tile([128, NT * 2 * D], bf16, tag="he")
            for t in range(NT):
                nc.gpsimd.indirect_dma_start(
                    out=he_all[:, t * 2 * D:(t + 1) * 2 * D],
                    out_offset=None, in_=tabs[:],
                    in_offset=bass.IndirectOffsetOnAxis(ap=ix_t[:, t:t + 1], axis=0))
            xT = pers.tile([128, F // 128, BC], bf16, tag="xT")
            fv = featT.rearrange("(a p) b -> p a b", p=128)
            nc.sync.dma_start(out=xT[:, 0, :], in_=fv[:, 0, :])
            wp1_t = pers.tile([128, 2 * D + D + 6], bf16, tag="wp1")
            nc.sync.dma_start(out=wp1_t[:], in_=wp1[:])
            nc.sync.dma_start(out=xT[:, 1, :], in_=fv[:, 1, :])
            wf_t = wp1_t[:, 0:2 * D].rearrange("p (a d) -> p a d", d=D)
            wu_t = wp1_t[:, 2 * D:3 * D]
            whe_t = wp1_t[:, 3 * D:3 * D + 6]
            identb = pers.tile([128, 128], bf16, tag="identb")
            nc.sync.dma_start(out=identb[:], in_=identb_d[:])
            bp_t = pers.tile([128, 11], f32, tag="bp")
            nc.sync.dma_start(out=bp_t[:], in_=bp[:])
            bfu_t = bp_t[:, 0:2]
            b0_t = bp_t[:, 2:6]
            b1_t = bp_t[:, 6:8]
            bce_t = bp_t[:, 8:10]
            b2_t = bp_t[:OUT, 10:11]
            wp2_t = pers.tile([128, 2 * H0 + 4 * H1 + 2 * OUT], bf16, tag="wp2")
            nc.sync.dma_start(out=wp2_t[:], in_=wp2[:])
            w0_t = wp2_t[:, 0:2 * H0].rearrange("p (a h) -> p a h", h=H0)
            w1_t = wp2_t[:, 2 * H0:2 * H0 + 4 * H1].rearrange(
                "p (a h) -> p a h", h=H1)
            w2_t = wp2_t[:, 2 * H0 + 4 * H1:].rearrange("p (a o) -> p a o", o=OUT)

            ones_t = pers.tile([128, 128], bf16, tag="ones_t")
            nc.vector.memset(ones_t[:], 1.0)

            # ---- column sums of wHE, broadcast to all partitions ----
            ones_c = pers.tile([128, 1], bf16, tag="ones_c")
            nc.vector.memset(ones_c[:], 1.0)
            ones_r = pers.tile([1, 128], bf16, tag="ones_r")
            nc.vector.memset(ones_r[:], 1.0)
            sums_ps = psp.tile([1, 6], f32, tag="smps", bufs=2)
            nc.tensor.matmul(out=sums_ps[:], lhsT=ones_c[:], rhs=whe_t,
                             start=True, stop=True)
            sums_sb = pers.tile([1, 6], bf16, tag="sums")
            nc.scalar.activation(out=sums_sb[:], in_=sums_ps[:], func=AF.Copy)
            sb_ps = psp.tile([128, 6], f32, tag="smps", bufs=2)
            nc.tensor.matmul(out=sb_ps[:], lhsT=ones_r[:], rhs=sums_sb[:],
                             start=True, stop=True)
            sb_t = pers.tile([128, 6], bf16, tag="sb")
            nc.scalar.activation(out=sb_t[:], in_=sb_ps[:], func=AF.Copy)
            # cef = b_e * sum(w_ef); cfe = b_c * sum(w_fe)
            cef = pers.tile([128, 1], f32, tag="cef")
            nc.vector.tensor_tensor(out=cef[:], in0=sb_t[:, 2:3], in1=bce_t[:, 1:2],
                                    op=OP.mult)
            cfe = pers.tile([128, 1], f32, tag="cfe")
            nc.vector.tensor_tensor(out=cfe[:], in0=sb_t[:, 1:2], in1=bce_t[:, 0:1],
                                    op=OP.mult)

            def h0s(t):
                return he_all[:, t * 2 * D:t * 2 * D + D]

            def e0s(t):
                return he_all[:, t * 2 * D + D:(t + 1) * 2 * D]

            hT_all = pers.tile([128, BC], bf16, tag="hT")
            eT_all = pers.tile([128, BC], bf16, tag="eT")
            dots = pers.tile([128, NT * 8], bf16, tag="dots")
            A = pers.tile([128, NT], f32, tag="A")
            Bc = pers.tile([128, NT], f32, tag="B")
            Cc = pers.tile([128, NT], f32, tag="C")
            e2T = pers.tile([128, BC], bf16, tag="e2T")

            def tt(out, a, b, op):
                nc.vector.tensor_tensor(out=out, in0=a, in1=b, op=op)

            def td(g):
                """transpose h0/e0 + dot products for one 512-group."""
                gs = slice(g * 512, (g + 1) * 512)
                for sel, dst in ((h0s, hT_all), (e0s, eT_all)):
                    tp = psp.tile([128, 512], bf16, tag="trps", bufs=2)
                    for j in range(GT):
                        nc.tensor.transpose(out=tp[:, j * 128:(j + 1) * 128],
                                            in_=sel(g * GT + j), identity=identb[:])
                    nc.scalar.activation(out=dst[:, gs], in_=tp[:], func=AF.Copy)
                # dots: 0..3 = h0.(w_fc,w_fe,w_ef,w_cf); 4..7 = e0.(w_ef,w_cf,w_ef,w_fe)
                for j in range(GT):
                    t = g * GT + j
                    bs = slice(t * 128, (t + 1) * 128)
                    d_ps = psp.tile([128, 8], f32, tag="smps", bufs=2)
                    nc.tensor.matmul(out=d_ps[:, 0:4], lhsT=hT_all[:, bs],
                                     rhs=whe_t[:, 0:4], start=True, stop=True)
                    nc.tensor.matmul(out=d_ps[:, 4:8], lhsT=eT_all[:, bs],
                                     rhs=whe_t[:, 2:6], start=True, stop=True)
                    nc.scalar.activation(out=dots[:, t * 8:(t + 1) * 8],
                                         in_=d_ps[:, 0:8], func=AF.Copy)
            def e2g(g):
                gs = slice(g * 512, (g + 1) * 512)
                tp = psp.tile([128, 512], bf16, tag="trps", bufs=2)
                for j in range(GT):
                    t = g * GT + j
                    m1 = work.tile([128, 128], bf16, tag="m1")
                    nc.scalar.activation(out=m1[:], in_=h0s(t), func=AF.Identity,
                                         scale=A[:, t:t + 1], bias=Cc[:, t:t + 1])
                    e2n = work.tile([128, 128], bf16, tag="e2n")
                    nc.vector.tensor_scalar(out=e2n[:], in0=e0s(t),
                                            scalar1=Bc[:, t:t + 1], scalar2=None,
                                            op0=OP.mult)
                    tt(e2n[:], m1[:], e2n[:], OP.add)
                    nc.tensor.transpose(out=tp[:, j * 128:(j + 1) * 128],
                                        in_=e2n[:], identity=identb[:])
                nc.scalar.activation(out=e2T[:, gs], in_=tp[:], func=AF.Copy)

            def coef():
                # coefficients for all 16 tiles at once
                dv = dots.rearrange("p (t c) -> p t c", c=8)
                c0, c1, c2 = dv[:, :, 0], dv[:, :, 1], dv[:, :, 2]
                c3, c4, c5 = dv[:, :, 5], dv[:, :, 4], dv[:, :, 7]
                t1 = work.tile([128, NT], f32, tag="t1")
                t2 = work.tile([128, NT], f32, tag="t2")
                a2 = work.tile([128, NT], f32, tag="a2")
                d2 = work.tile([128, NT], f32, tag="d2")
                # a2 = c4*c2 + c1*c4 + cef ; d2 = c3*c1 + c0*c5 + cfe
                tt(t1[:], c4, c2, OP.mult)
                tt(t2[:], c1, c4, OP.mult)
                tt(a2[:], t1[:], t2[:], OP.add)
                nc.vector.tensor_scalar(out=a2[:], in0=a2[:], scalar1=cef[:, 0:1],
                                        scalar2=None, op0=OP.add)
                tt(t1[:], c3, c1, OP.mult)
                tt(t2[:], c0, c5, OP.mult)
                tt(d2[:], t1[:], t2[:], OP.add)
                nc.vector.tensor_scalar(out=d2[:], in0=d2[:], scalar1=cfe[:, 0:1],
                                        scalar2=None, op0=OP.add)
                # A = a2*c3 + d2*c4 ; B = a2*c0 + d2*c1 ; C = a2*b_c + (d2*b_e + b_e)
                tt(t1[:], a2[:], c3, OP.mult)
                tt(t2[:], d2[:], c4, OP.mult)
                tt(A[:], t1[:], t2[:], OP.add)
                tt(t1[:], a2[:], c0, OP.mult)
                tt(t2[:], d2[:], c1, OP.mult)
                tt(Bc[:], t1[:], t2[:], OP.add)
                nc.vector.tensor_scalar(out=t1[:], in0=a2[:], scalar1=bce_t[:, 0:1],
                                        scalar2=None, op0=OP.mult)
                nc.vector.tensor_scalar(out=t2[:], in0=d2[:], scalar1=bce_t[:, 1:2],
                                        scalar2=bce_t[:, 1:2], op0=OP.mult, op1=OP.add)
                tt(Cc[:], t1[:], t2[:], OP.add)

            GS = [slice(g * 512, (g + 1) * 512) for g in range(NG)]

            def relu(dst, src, bias_ap, on_vector):
                if on_vector:
                    nc.vector.tensor_scalar(out=dst, in0=src, scalar1=bias_ap,
                                            scalar2=0.0, op0=OP.add, op1=OP.max)
                else:
                    nc.scalar.activation(out=dst, in_=src, func=AF.Relu, bias=bias_ap)

            cf0 = pers.tile([128, BC], bf16, tag="cf0")
            cf1 = pers.tile([128, BC], bf16, tag="cf1")
            cf2 = pers.tile([128, BC], bf16, tag="cf2")
            x1a = pers.tile([128, NG, 4 * 512], bf16, tag="x1a")
            x2a = pers.tile([128, NG, 2 * 512], bf16, tag="x2a")
            x3a = pers.tile([OUT, NG, 512], bf16, tag="x3a")

            def mlp0(g):
                """feature layer + 2x user mlp for one group (needs only xT)."""
                cf_ps = psp.tile([128, 512], f32, tag="mmps", bufs=4)
                nc.tensor.matmul(out=cf_ps[:], lhsT=wf_t[:, 0, :], rhs=xT[:, 0, GS[g]],
                                 start=True, stop=False)
                nc.tensor.matmul(out=cf_ps[:], lhsT=wf_t[:, 1, :], rhs=xT[:, 1, GS[g]],
                                 start=False, stop=True)
                relu(cf0[:, GS[g]], cf_ps[:], bfu_t[:, 0:1], on_vector=False)
                for src, dst in ((cf0, cf1), (cf1, cf2)):
                    cu_ps = psp.tile([128, 512], f32, tag="mmps", bufs=4)
                    nc.tensor.matmul(out=cu_ps[:], lhsT=wu_t, rhs=src[:, GS[g]],
                                     start=True, stop=True)
                    relu(dst[:, GS[g]], cu_ps[:], bfu_t[:, 1:2],
                         on_vector=(g % 2 == 1))

            def w0(g):
                for m in range(4):
                    x1_ps = psp.tile([128, 512], f32, tag="mmps", bufs=4)
                    ms = slice(m * 128, (m + 1) * 128)
                    nc.tensor.matmul(out=x1_ps[:], lhsT=w0_t[:, 0, ms],
                                     rhs=cf2[:, GS[g]], start=True, stop=False)
                    nc.tensor.matmul(out=x1_ps[:], lhsT=w0_t[:, 1, ms],
                                     rhs=e2T[:, GS[g]], start=False, stop=True)
                    relu(x1a[:, g, m * 512:(m + 1) * 512], x1_ps[:], b0_t[:, m:m + 1],
                         on_vector=(m % 2 == 1))

            def w1(g):
                for m in range(2):
                    x2_ps = psp.tile([128, 512], f32, tag="mmps", bufs=4)
                    ms = slice(m * 128, (m + 1) * 128)
                    for k in range(4):
                        nc.tensor.matmul(out=x2_ps[:], lhsT=w1_t[:, k, ms],
                                         rhs=x1a[:, g, k * 512:(k + 1) * 512],
                                         start=(k == 0), stop=(k == 3))
                    relu(x2a[:, g, m * 512:(m + 1) * 512], x2_ps[:], b1_t[:, m:m + 1],
                         on_vector=(m % 2 == 1))

            def w2sm(g):
                """pred layer + softmax + store for one group."""
                x3_ps = psp.tile([OUT, 512], f32, tag="smps", bufs=2)
                for k in range(2):
                    nc.tensor.matmul(out=x3_ps[:], lhsT=w2_t[:, k, :],
                                     rhs=x2a[:, g, k * 512:(k + 1) * 512],
                                     start=(k == 0), stop=(k == 1))
                nc.scalar.activation(out=x3a[:, g, :], in_=x3_ps[:], func=AF.Relu,
                                     bias=b2_t[:, 0:1])
                ex = work.tile([128, GT * OUT], f32, tag="ex")
                for j in range(GT):
                    sm_ps = psp.tile([128, OUT], bf16, tag="smps", bufs=2)
                    nc.tensor.transpose(out=sm_ps[:],
                                        in_=x3a[:, g, j * 128:(j + 1) * 128],
                                        identity=identb[:OUT, :OUT])
                    nc.scalar.activation(out=ex[:, j * OUT:(j + 1) * OUT],
                                         in_=sm_ps[:], func=AF.Exp)
                sm_sum = work.tile([128, GT], f32, tag="sm_sum")
                nc.vector.reduce_sum(out=sm_sum[:],
                                     in_=ex.rearrange("p (j o) -> p j o", o=OUT),
                                     axis=AX.X)
                sm_rec = work.tile([128, GT], f32, tag="sm_rec")
                nc.vector.reciprocal(out=sm_rec[:], in_=sm_sum[:])
                pr = work.tile([128, GT * OUT], f32, tag="pr")
                nc.vector.tensor_tensor(
                    out=pr.rearrange("p (j o) -> p j o", o=OUT),
                    in0=ex.rearrange("p (j o) -> p j o", o=OUT),
                    in1=sm_rec.rearrange("p (j o) -> p j o", o=1).to_broadcast(
                        [128, GT, OUT]),
                    op=OP.mult)
                nc.sync.dma_start(
                    out=prob[g * 512:(g + 1) * 512, :].rearrange(
                        "(j p) o -> p j o", p=128),
                    in_=pr.rearrange("p (j o) -> p j o", o=OUT))

            # PE program order: early group-independent matmuls first, then
            # cross-compress chains interleaved with later-stage MLP work so
            # the PE never stalls on a not-yet-gathered group.
            for g in range(NG):
                mlp0(g)
            td(0)
            td(1)
            td(2)
            td(3)
            coef()
            e2g(0)
            w0(0)
            e2g(1)
            w0(1)
            w1(0)
            e2g(2)
            w0(2)
            w1(1)
            w2sm(0)
            e2g(3)
            w0(3)
            w1(2)
            w2sm(1)
            w1(3)
            w2sm(2)
            w2sm(3)

    nc.finalize()
    return nc


def _get_nc():
    if "nc" not in _CACHE:
        _CACHE["nc"] = _build()
    return _CACHE["nc"]


def kernel(features, ent_idx, target, Wf, bf, Wu, bu, w_cf, w_fc, w_ef, w_fe,
           b_c, b_e, head_tab, ent_tab, W0, b0, W1, b1, W2, b2):
    import ml_dtypes
    from concourse.bass_utils import run_bass_kernel_spmd

    bf16 = ml_dtypes.bfloat16
    f32 = np.float32
    features = np.asarray(features, dtype=f32)
    ent_idx = np.asarray(ent_idx)
    target = np.asarray(target)
    head_tab = np.asarray(head_tab, dtype=f32)
    ent_tab = np.asarray(ent_tab, dtype=f32)
    w_cf, w_fc = np.asarray(w_cf, f32), np.asarray(w_fc, f32)
    w_ef, w_fe = np.asarray(w_ef, f32), np.asarray(w_fe, f32)

    featT = np.ascontiguousarray(features.T.astype(bf16))        # [F, B]
    idx_all = ent_idx.astype(np.int32)
    tabs = np.ascontiguousarray(np.concatenate([head_tab, ent_tab], axis=1))
    wHE = np.stack([w_fc, w_fe, w_ef, w_cf, w_ef, w_fe], axis=1).astype(bf16)
    wf_p = np.asarray(Wf, f32).astype(bf16).reshape(2, D, D).transpose(
        1, 0, 2).reshape(D, 2 * D)
    wu_p = np.asarray(Wu, f32).astype(bf16)
    wp1 = np.ascontiguousarray(np.concatenate([wf_p, wu_p, wHE], axis=1))
    w0_p = np.asarray(W0, f32).astype(bf16).reshape(2, D, H0).transpose(
        1, 0, 2).reshape(D, 2 * H0)
    w1_p = np.asarray(W1, f32).astype(bf16).reshape(4, D, H1).transpose(
        1, 0, 2).reshape(D, 4 * H1)
    w2_p = np.asarray(W2, f32).astype(bf16).reshape(2, D, OUT).transpose(
        1, 0, 2).reshape(D, 2 * OUT)
    wp2 = np.ascontiguousarray(np.concatenate([w0_p, w1_p, w2_p], axis=1))
    bp = np.zeros((128, 11), f32)
    bp[:, 0] = np.asarray(bf, f32)
    bp[:, 1] = np.asarray(bu, f32)
    bp[:, 2:6] = np.asarray(b0, f32).reshape(4, D).T
    bp[:, 6:8] = np.asarray(b1, f32).reshape(2, D).T
    bp[:, 8] = np.float32(np.asarray(b_c, f32).reshape(()))
    bp[:, 9] = np.float32(np.asarray(b_e, f32).reshape(()))
    bp[:OUT, 10] = np.asarray(b2, f32)
    identb = np.eye(128, dtype=bf16)

    shared = dict(tabs=tabs, identb=identb, wp1=wp1, wp2=wp2, bp=bp)
    in_maps = []
    for c in range(NCORES):
        cs = slice(c * BC, (c + 1) * BC)
        in_maps.append(dict(
            featT=np.ascontiguousarray(featT[:, cs]),
            idx2=np.ascontiguousarray(idx_all[cs].reshape(NT, 128).T),
            **shared))

    nc = _get_nc()
    res = run_bass_kernel_spmd(nc, in_maps, core_ids=list(range(NCORES)))
    prob = np.concatenate([r["prob"] for r in res.results], axis=0)
    return prob, target
